# revision 21
# baseline (speedup 1.0000x reference)
"""CPA-loss kernel for 8 TRN2 NeuronCores.

Math: for row b with target t, the reference loss collapses to
    loss[b] = -log( e[b,t] / (dot(s[t,:], e[b,:]) + eps) + eps ),
    e = exp(z - max(z))  (the s[t,t]=1 diagonal cancels the "+e[b,i]" term).
Both e[b,t] and the dot are invariant to the max-subtraction except for the
eps scale (effect ~2e-7 on the mean loss, far below fp32 tolerance), so we
use e = exp(z) directly and never compute the row max.

Strategy: sort rows by target on the host (the mean is permutation
invariant), deal them round-robin to 8 cores. Each core gets its 16384 rows
as a transposed [100, 16384] tile (class on partitions). Consecutive sorted
rows share targets, so every 128-row block touches at most a few distinct
classes; per block we ship the m candidate s[c,:] columns and do ONE PE
matmul  out[128, m] = (E^T block [100,128]).T @ V[100, m]  giving every
row's candidate denominator dot. Host-built 0/1 masks select each row's
true class candidate. The numerator e[b,t] = exp(logits[b, t_b]) comes from
a host-gathered z_t column (pure index selection), exp'd on device. A short
batched DVE/ACT epilogue computes -log(e_t/(D+eps)+eps) and reduces.
"""

import sys

import ml_dtypes
import numpy as np

for _p in ("/opt/trn_rl_repo",):
    if _p not in sys.path:
        sys.path.append(_p)

import concourse.bass as bass
import concourse.tile as tile
from concourse import bacc, mybir
from concourse.bass_utils import run_bass_kernel_spmd

B = 131072
C = 100
NCORES = 8
RPC = B // NCORES  # 16384 rows per core
BLK = 128  # rows per block (= one matmul stationary tile)
NBLK = RPC // BLK  # 128 blocks per core
# DMA/exp chunk sizes in blocks: small first groups fill the pipeline fast
GSIZES = [4, 4, 8] + [16] * 7
EPS = 1e-6

TRACE = False  # test.py flips this to get a profiled run
LAST_RESULTS = None  # stash of the last BassKernelResults (for test.py)

_nc_cache = {}


def _build_nc(m: int, stride: int):
    """Trace the SPMD program. m = candidate s-columns per block, stride =
    padded candidate stride in the PSUM result tile (divides 512)."""
    nc = bacc.Bacc("TRN2", target_bir_lowering=False, debug=False)
    f32 = mybir.dt.float32
    f32r = mybir.dt.float32r

    # group-major contiguous layout: each chunk is one sequential DRAM read
    lt_d = nc.declare_dram_parameter("lt", [C * RPC], mybir.dt.float16, isOutput=False)
    vs_d = nc.declare_dram_parameter("vs", [C, m * NBLK], mybir.dt.float16, isOutput=False)
    zt_d = nc.declare_dram_parameter("zt", [BLK, NBLK], f32, isOutput=False)
    w_d = [
        nc.declare_dram_parameter(f"w{i}", [BLK, NBLK], mybir.dt.uint8, isOutput=False)
        for i in range(max(m - 1, 1))
    ]
    out_d = nc.declare_dram_parameter("out", [BLK, 1], f32, isOutput=True)

    # epilogue is emitted in SLICES column-slices, each with its own PSUM
    # bank so it overlaps the matmul stream.
    gsizes = GSIZES
    assert sum(gsizes) == NBLK
    SLICES = 8
    SBLK = NBLK // SLICES

    with tile.TileContext(nc) as tc:
        with (
            tc.tile_pool(name="const", bufs=1) as cpool,
            tc.tile_pool(name="lt", bufs=4) as ltp,
            tc.tile_pool(name="et", bufs=3) as etp,
            tc.tile_pool(name="fin", bufs=1) as fin,
            tc.tile_pool(name="res", bufs=1, space="PSUM") as resp,
        ):
            def lt_slice(g):
                off = C * BLK * sum(gsizes[:g])
                n = C * gsizes[g] * BLK
                return lt_d[off : off + n].rearrange("(j c) -> j c", j=C)

            # first logits chunk before anything else
            lt0 = ltp.tile([C, gsizes[0] * BLK], mybir.dt.float16, tag="lt")
            nc.sync.dma_start(lt0[:], lt_slice(0))
            vs_sb = cpool.tile([C, m * NBLK], mybir.dt.float16)
            nc.scalar.dma_start(vs_sb[:], vs_d[:])
            zt_sb = cpool.tile([BLK, NBLK], f32)
            nc.sync.dma_start(zt_sb[:], zt_d[:])
            w_sb = []
            for i in range(max(m - 1, 1)):
                w = cpool.tile([BLK, NBLK], mybir.dt.uint8, tag=f"w{i}")
                nc.sync.dma_start(w[:], w_d[i][:])
                w_sb.append(w)

            res = [
                resp.tile([BLK, SBLK, stride], f32, tag=f"res{i}", name=f"res{i}")
                for i in range(SLICES)
            ]
            rp_full = fin.tile([BLK, NBLK], f32)
            et_full = fin.tile([BLK, NBLK], f32)
            nc.scalar.activation(
                et_full[:], zt_sb[:], mybir.ActivationFunctionType.Exp
            )

            def epilogue(sl):
                """select candidate, exp(zt)/(D+eps)+eps for slice sl.
                Ln happens once at the end (one ACT table switch)."""
                cols = slice(sl * SBLK, (sl + 1) * SBLK)
                rsl = res[sl]
                dsel = fin.tile([BLK, SBLK], f32, tag="dsel")
                if m == 1:
                    nc.vector.tensor_copy(dsel[:], rsl[:, :, 0])
                else:
                    nc.vector.tensor_copy(dsel[:], rsl[:, :, m - 1])
                    for i in range(m - 2, -1, -1):
                        nc.vector.copy_predicated(
                            dsel[:], w_sb[i][:, cols], rsl[:, :, i]
                        )
                if sl == SLICES - 1:
                    # tiny dummy Ln: pulls the ACT table switch off the
                    # critical tail (the real Ln then needs no reload)
                    dummy = fin.tile([1, 1], f32, tag="dummy")
                    nc.scalar.activation(
                        dummy[:], zt_sb[0:1, 0:1], mybir.ActivationFunctionType.Ln
                    )
                dp = fin.tile([BLK, SBLK], f32, tag="dp")
                nc.vector.tensor_scalar_add(dp[:], dsel[:], EPS)
                rec = fin.tile([BLK, SBLK], f32, tag="rec")
                nc.vector.reciprocal(rec[:], dp[:])
                r = fin.tile([BLK, SBLK], f32, tag="r")
                nc.vector.tensor_tensor(
                    r[:], et_full[:, cols], rec[:], op=mybir.AluOpType.mult
                )
                nc.vector.tensor_scalar_add(rp_full[:, cols], r[:], EPS)

            # spread the logits loads across the two HWDGE queues (sync +
            # scalar) — one queue alone is ~150 GB/s. gpsimd SWDGE is avoided
            # entirely: its kernel-exit dge_drain costs ~8us once used.
            scalar_groups = {5, 8}
            kk = 0
            done = 0
            for g, gs in enumerate(gsizes):
                base = sum(gsizes[:g]) * BLK
                if g == 0:
                    ltg = lt0
                else:
                    ltg = ltp.tile([C, gs * BLK], mybir.dt.float16, tag="lt")
                    eng = nc.scalar if g in scalar_groups else nc.sync
                    eng.dma_start(ltg[:], lt_slice(g))
                etg = etp.tile([C, gs * BLK], mybir.dt.float16, tag="et")
                nc.scalar.activation(
                    etg[:], ltg[:], mybir.ActivationFunctionType.Exp
                )
                for k in range(gs):
                    sl, j = kk // SBLK, kk % SBLK
                    nc.tensor.matmul(
                        res[sl][:, j, 0:m],
                        etg[:, k * BLK : (k + 1) * BLK],
                        vs_sb[:, m * kk : m * (kk + 1)],
                        start=True,
                        stop=True,
                    )
                    kk += 1
                while done < SLICES and kk >= (done + 1) * SBLK:
                    epilogue(done)
                    done += 1
            while done < SLICES:
                epilogue(done)
                done += 1

            lnr = fin.tile([BLK, NBLK], f32)
            lsum = fin.tile([BLK, 1], f32)
            nc.scalar.activation(
                lnr[:],
                rp_full[:],
                mybir.ActivationFunctionType.Ln,
                accum_out=lsum[:],
            )
            nc.sync.dma_start(out_d[:], lsum[:])

    nc.compile()
    return nc


def _pick_stride(m: int) -> int:
    # candidate-group stride in f32 elems; must divide the 512-f32 PSUM bank
    for st in (1, 2, 4, 8, 16):
        if st >= m and 512 % st == 0:
            return st
    raise ValueError(f"too many classes per block: m={m}")


def kernel(logits, s, targets):
    global LAST_RESULTS
    logits = np.asarray(logits, dtype=np.float32)
    s = np.asarray(s, dtype=np.float32)
    t = np.asarray(targets).astype(np.int64).ravel()
    assert logits.shape == (B, C) and s.shape == (C, C) and t.shape == (B,)

    order = np.argsort(t, kind="stable")
    zt_all = logits[np.arange(B), t]  # host gather of logits[b, t_b]

    # per-core index sets (round-robin over globally sorted rows)
    idxs = [order[mm::NCORES] for mm in range(NCORES)]

    # classes per block: blocks are rows [128k, 128(k+1)) of the sorted core
    # slice; count the max distinct classes any block touches
    m = 1
    block_classes = []
    for idx in idxs:
        tb = t[idx].reshape(NBLK, BLK)
        cs = [np.unique(row) for row in tb]
        m = max(m, max(len(u) for u in cs))
        block_classes.append((tb, cs))
    stride = _pick_stride(m)

    in_maps = []
    for core in range(NCORES):
        idx = idxs[core]
        tb, cs = block_classes[core]
        ltT = logits[idx].T.astype(np.float16)  # [100, 16384]
        bounds = np.cumsum([0] + GSIZES) * BLK
        lt = np.concatenate(
            [ltT[:, a:b].ravel() for a, b in zip(bounds[:-1], bounds[1:])]
        )
        zt = np.ascontiguousarray(zt_all[idx].reshape(NBLK, BLK).T)  # [BLK,NBLK]
        vs = np.empty((C, m * NBLK), dtype=np.float16)
        cmat = np.empty((m, NBLK), dtype=np.int64)
        for k in range(NBLK):
            u = cs[k]
            cmat[: len(u), k] = u
            cmat[len(u) :, k] = u[-1]
        for i in range(m):
            vs[:, i::m] = s[cmat[i]].T.astype(np.float16)
        im = {"lt": lt, "vs": vs, "zt": zt}
        nw = max(m - 1, 1)
        for i in range(nw):
            wi = (tb == cmat[i][:, None]).T.astype(np.uint8)  # [BLK, NBLK]
            im[f"w{i}"] = np.ascontiguousarray(wi)
        in_maps.append(im)

    key = (m, stride)
    if key not in _nc_cache:
        _nc_cache[key] = _build_nc(m, stride)
    nc = _nc_cache[key]

    res = run_bass_kernel_spmd(
        nc, in_maps, core_ids=list(range(NCORES)), trace=TRACE
    )
    LAST_RESULTS = res
    total = sum(float(r["out"].sum(dtype=np.float64)) for r in res.results)
    return np.float32(-total / B)


# revision 22
# speedup vs baseline: 1.1788x; 1.1788x over previous
"""CPA-loss kernel for 8 TRN2 NeuronCores.

Math: for row b with target t, the reference loss collapses to
    loss[b] = -log( e[b,t] / (dot(s[t,:], e[b,:]) + eps) + eps ),
    e = exp(z - max(z))  (the s[t,t]=1 diagonal cancels the "+e[b,i]" term).
Both e[b,t] and the dot are invariant to the max-subtraction except for the
eps scale (effect ~2e-7 on the mean loss, far below fp32 tolerance), so we
use e = exp(z) directly and never compute the row max.

Strategy: sort rows by target on the host (the mean is permutation
invariant), deal them round-robin to 8 cores. Each core gets its 16384 rows
as a transposed [100, 16384] tile (class on partitions). Consecutive sorted
rows share targets, so every 128-row block touches at most a few distinct
classes; per block we ship the m candidate s[c,:] columns and do ONE PE
matmul  out[128, m] = (E^T block [100,128]).T @ V[100, m]  giving every
row's candidate denominator dot. Host-built 0/1 masks select each row's
true class candidate. The numerator e[b,t] = exp(logits[b, t_b]) comes from
a host-gathered z_t column (pure index selection), exp'd on device. A short
batched DVE/ACT epilogue computes -log(e_t/(D+eps)+eps) and reduces.
"""

import sys

import ml_dtypes
import numpy as np

for _p in ("/opt/trn_rl_repo",):
    if _p not in sys.path:
        sys.path.append(_p)

import concourse.bass as bass
import concourse.tile as tile
from concourse import bacc, mybir
from concourse.bass_utils import run_bass_kernel_spmd

B = 131072
C = 100
NCORES = 8
RPC = B // NCORES  # 16384 rows per core
BLK = 128  # rows per block (= one matmul stationary tile)
NBLK = RPC // BLK  # 128 blocks per core
# DMA/exp chunk sizes in blocks: small first groups fill the pipeline fast
GSIZES = [4, 4, 8] + [16] * 7
EPS = 1e-6

TRACE = False  # test.py flips this to get a profiled run
LAST_RESULTS = None  # stash of the last BassKernelResults (for test.py)

_nc_cache = {}


def _build_nc(m: int, stride: int):
    """Trace the SPMD program. m = candidate s-columns per block, stride =
    padded candidate stride in the PSUM result tile (divides 512)."""
    nc = bacc.Bacc("TRN2", target_bir_lowering=False, debug=False)
    f32 = mybir.dt.float32
    f32r = mybir.dt.float32r

    # group-major contiguous layout: each chunk is one sequential DRAM read
    lt_d = nc.declare_dram_parameter("lt", [C * RPC], mybir.dt.float16, isOutput=False)
    vs_d = nc.declare_dram_parameter("vs", [C, m * NBLK], mybir.dt.float16, isOutput=False)
    zt_d = nc.declare_dram_parameter("zt", [BLK, NBLK], f32, isOutput=False)
    w_d = [
        nc.declare_dram_parameter(f"w{i}", [BLK, NBLK], mybir.dt.uint8, isOutput=False)
        for i in range(max(m - 1, 1))
    ]
    out_d = nc.declare_dram_parameter("out", [BLK, 1], f32, isOutput=True)

    # epilogue is emitted in SLICES column-slices, each with its own PSUM
    # bank so it overlaps the matmul stream.
    gsizes = GSIZES
    assert sum(gsizes) == NBLK
    SLICES = 8
    SBLK = NBLK // SLICES

    with tile.TileContext(nc) as tc:
        with (
            tc.tile_pool(name="const", bufs=1) as cpool,
            tc.tile_pool(name="lt", bufs=4) as ltp,
            tc.tile_pool(name="et", bufs=3) as etp,
            tc.tile_pool(name="fin", bufs=1) as fin,
            tc.tile_pool(name="res", bufs=1, space="PSUM") as resp,
        ):
            def lt_slice(g):
                off = C * BLK * sum(gsizes[:g])
                n = C * gsizes[g] * BLK
                return lt_d[off : off + n].rearrange("(j c) -> j c", j=C)

            # first logits chunk before anything else
            lt0 = ltp.tile([C, gsizes[0] * BLK], mybir.dt.float16, tag="lt")
            nc.sync.dma_start(lt0[:], lt_slice(0))
            vs_sb = cpool.tile([C, m * NBLK], mybir.dt.float16)
            nc.scalar.dma_start(vs_sb[:], vs_d[:])
            zt_sb = cpool.tile([BLK, NBLK], f32)
            nc.sync.dma_start(zt_sb[:], zt_d[:])
            w_sb = []
            for i in range(max(m - 1, 1)):
                w = cpool.tile([BLK, NBLK], mybir.dt.uint8, tag=f"w{i}")
                nc.sync.dma_start(w[:], w_d[i][:])
                w_sb.append(w)

            res = [
                resp.tile([BLK, SBLK, stride], f32, tag=f"res{i}", name=f"res{i}")
                for i in range(SLICES)
            ]
            rp_full = fin.tile([BLK, NBLK], f32)
            et_full = fin.tile([BLK, NBLK], f32)
            nc.scalar.activation(
                et_full[:], zt_sb[:], mybir.ActivationFunctionType.Exp
            )

            def epilogue(sl):
                """select candidate, exp(zt)/(D+eps)+eps for slice sl.
                Ln happens once at the end (one ACT table switch)."""
                cols = slice(sl * SBLK, (sl + 1) * SBLK)
                rsl = res[sl]
                dsel = fin.tile([BLK, SBLK], f32, tag="dsel")
                if m == 1:
                    nc.vector.tensor_copy(dsel[:], rsl[:, :, 0])
                else:
                    nc.vector.tensor_copy(dsel[:], rsl[:, :, m - 1])
                    for i in range(m - 2, -1, -1):
                        nc.vector.copy_predicated(
                            dsel[:], w_sb[i][:, cols], rsl[:, :, i]
                        )
                if sl == SLICES - 1:
                    # tiny dummy Ln: pulls the ACT table switch off the
                    # critical tail (the real Ln then needs no reload)
                    dummy = fin.tile([1, 1], f32, tag="dummy")
                    nc.scalar.activation(
                        dummy[:], zt_sb[0:1, 0:1], mybir.ActivationFunctionType.Ln
                    )
                dp = fin.tile([BLK, SBLK], f32, tag="dp")
                nc.vector.tensor_scalar_add(dp[:], dsel[:], EPS)
                rec = fin.tile([BLK, SBLK], f32, tag="rec")
                nc.vector.reciprocal(rec[:], dp[:])
                r = fin.tile([BLK, SBLK], f32, tag="r")
                nc.vector.tensor_tensor(
                    r[:], et_full[:, cols], rec[:], op=mybir.AluOpType.mult
                )
                nc.vector.tensor_scalar_add(rp_full[:, cols], r[:], EPS)

            # spread the logits loads across the two HWDGE queues (sync +
            # scalar) — one queue alone is ~150 GB/s. gpsimd SWDGE is avoided
            # entirely: its kernel-exit dge_drain costs ~8us once used.
            dma_engines = [nc.sync, nc.gpsimd]
            kk = 0
            done = 0
            for g, gs in enumerate(gsizes):
                base = sum(gsizes[:g]) * BLK
                if g == 0:
                    ltg = lt0
                else:
                    ltg = ltp.tile([C, gs * BLK], mybir.dt.float16, tag="lt")
                    eng = dma_engines[g % len(dma_engines)]
                    eng.dma_start(ltg[:], lt_slice(g))
                etg = etp.tile([C, gs * BLK], mybir.dt.float16, tag="et")
                nc.scalar.activation(
                    etg[:], ltg[:], mybir.ActivationFunctionType.Exp
                )
                for k in range(gs):
                    sl, j = kk // SBLK, kk % SBLK
                    nc.tensor.matmul(
                        res[sl][:, j, 0:m],
                        etg[:, k * BLK : (k + 1) * BLK],
                        vs_sb[:, m * kk : m * (kk + 1)],
                        start=True,
                        stop=True,
                    )
                    kk += 1
                while done < SLICES and kk >= (done + 1) * SBLK:
                    epilogue(done)
                    done += 1
            while done < SLICES:
                epilogue(done)
                done += 1

            lnr = fin.tile([BLK, NBLK], f32)
            lsum = fin.tile([BLK, 1], f32)
            nc.scalar.activation(
                lnr[:],
                rp_full[:],
                mybir.ActivationFunctionType.Ln,
                accum_out=lsum[:],
            )
            nc.sync.dma_start(out_d[:], lsum[:])

    nc.compile()
    return nc


def _pick_stride(m: int) -> int:
    # candidate-group stride in f32 elems; must divide the 512-f32 PSUM bank
    for st in (1, 2, 4, 8, 16):
        if st >= m and 512 % st == 0:
            return st
    raise ValueError(f"too many classes per block: m={m}")


def kernel(logits, s, targets):
    global LAST_RESULTS
    logits = np.asarray(logits, dtype=np.float32)
    s = np.asarray(s, dtype=np.float32)
    t = np.asarray(targets).astype(np.int64).ravel()
    assert logits.shape == (B, C) and s.shape == (C, C) and t.shape == (B,)

    order = np.argsort(t, kind="stable")
    zt_all = logits[np.arange(B), t]  # host gather of logits[b, t_b]

    # per-core index sets (round-robin over globally sorted rows)
    idxs = [order[mm::NCORES] for mm in range(NCORES)]

    # classes per block: blocks are rows [128k, 128(k+1)) of the sorted core
    # slice; count the max distinct classes any block touches
    m = 1
    block_classes = []
    for idx in idxs:
        tb = t[idx].reshape(NBLK, BLK)
        cs = [np.unique(row) for row in tb]
        m = max(m, max(len(u) for u in cs))
        block_classes.append((tb, cs))
    stride = _pick_stride(m)

    in_maps = []
    for core in range(NCORES):
        idx = idxs[core]
        tb, cs = block_classes[core]
        ltT = logits[idx].T.astype(np.float16)  # [100, 16384]
        bounds = np.cumsum([0] + GSIZES) * BLK
        lt = np.concatenate(
            [ltT[:, a:b].ravel() for a, b in zip(bounds[:-1], bounds[1:])]
        )
        zt = np.ascontiguousarray(zt_all[idx].reshape(NBLK, BLK).T)  # [BLK,NBLK]
        vs = np.empty((C, m * NBLK), dtype=np.float16)
        cmat = np.empty((m, NBLK), dtype=np.int64)
        for k in range(NBLK):
            u = cs[k]
            cmat[: len(u), k] = u
            cmat[len(u) :, k] = u[-1]
        for i in range(m):
            vs[:, i::m] = s[cmat[i]].T.astype(np.float16)
        im = {"lt": lt, "vs": vs, "zt": zt}
        nw = max(m - 1, 1)
        for i in range(nw):
            wi = (tb == cmat[i][:, None]).T.astype(np.uint8)  # [BLK, NBLK]
            im[f"w{i}"] = np.ascontiguousarray(wi)
        in_maps.append(im)

    key = (m, stride)
    if key not in _nc_cache:
        _nc_cache[key] = _build_nc(m, stride)
    nc = _nc_cache[key]

    res = run_bass_kernel_spmd(
        nc, in_maps, core_ids=list(range(NCORES)), trace=TRACE
    )
    LAST_RESULTS = res
    total = sum(float(r["out"].sum(dtype=np.float64)) for r in res.results)
    return np.float32(-total / B)
